# revision 12
# baseline (speedup 1.0000x reference)
"""MoNet GNN message passing on 8 Trainium2 NeuronCores (Bass/Tile).

Sharding: graphs are partitioned across the 8 cores (7,7,6,6,6,6,6,6); each
core holds whole graphs (nodes+edges), small parameters are replicated.
Per layer, on each core:
  gw      = gaussian kernel weights per edge (from degrees + baked params)
  h_g     = dma_gather of h rows by edge src                [128e x 128H]
  agg^T   = sum over edge tiles of h_g^T @ (onehot(dst)*gw) [128H x 3W] PSUM
  msg^T   = sum_k fcW_k^T @ agg_k^T                         [128H x W]
  BN stats AllReduce over all 8 cores, then  h' = h + relu(bn(msg))
Readout: per-graph mean + 3-layer MLP, all on-device.
"""
import os
import sys

sys.path.insert(0, "/opt/trn_rl_repo")

import numpy as np

import concourse.bass as bass
import concourse.tile as tile
from concourse import bacc, mybir
from concourse.bass import AP
from concourse.bass_utils import run_bass_kernel_spmd
from concourse.masks import make_identity

N_CORES = 8
N = 50000
G = 50
NPG = 1000          # nodes per graph
H = 128             # hidden dim
K = 3               # gaussian kernels
L = 4               # layers
SLOT = 1024         # node slot per graph (1000 valid + 24 pad)
GPC = [7, 7, 6, 6, 6, 6, 6, 6]              # graphs per core
G0 = np.concatenate([[0], np.cumsum(GPC)])  # graph range starts
MAXG = 7
W = int(os.environ.get("KW", "96"))      # node window per aggregation block
NPC = {96: 7296, 64: 7168, 128: 7168}[W]
NB = NPC // W                            # aggregation blocks per core
GCHUNK = 1024                            # edges per dma_gather
EPS_BN = 1e-5
MM_DT = os.environ.get("KMM", "f32r")    # f32r | f32 | bf16
F32 = mybir.dt.float32
F32R = mybir.dt.float32r
BF16 = mybir.dt.bfloat16
AX = mybir.AxisListType
OP = mybir.AluOpType
AF = mybir.ActivationFunctionType


def _roundup(x, m):
    return (x + m - 1) // m * m


def _prepare(inputs):
    x = np.ascontiguousarray(inputs["x"], dtype=np.float32)
    src = np.asarray(inputs["src"]).astype(np.int64)
    dst = np.asarray(inputs["dst"]).astype(np.int64)
    assert x.shape == (N, H)

    gcore = np.zeros(G, np.int64)
    for c in range(N_CORES):
        gcore[G0[c]:G0[c + 1]] = c
    gslot = np.arange(G) - G0[gcore]

    node_g = np.arange(N) // NPG
    node_loc = gslot[node_g] * SLOT + (np.arange(N) % NPG)

    e_g_dst = dst // NPG
    assert np.array_equal(e_g_dst, src // NPG), "edges must stay within graphs"
    ecore = gcore[e_g_dst]

    deg = np.bincount(dst, minlength=N).astype(np.float32)

    per_core = []
    cnts = np.zeros((N_CORES, NB), np.int64)
    for c in range(N_CORES):
        sel = np.nonzero(ecore == c)[0]
        ls = node_loc[src[sel]]
        ld = node_loc[dst[sel]]
        blk = ld // W
        order = np.argsort(blk, kind="stable")
        ls, ld, blk = ls[order], ld[order], blk[order]
        cnts[c] = np.bincount(blk, minlength=NB)
        per_core.append((ls, ld, deg[src[sel][order]], deg[dst[sel][order]]))

    padded = np.maximum(128, _roundup(cnts.max(axis=0), 128))
    EP = int(padded.sum())
    extra = _roundup(EP, GCHUNK) - EP
    padded[-1] += extra
    EP += extra
    tstart = np.concatenate([[0], np.cumsum(padded // 128)]).astype(np.int64)
    EPQ = EP // 128
    EPS16 = EP // 16

    in_maps = []
    for c in range(N_CORES):
        ls, ld, dgs, dgd = per_core[c]
        e_src = np.zeros(EP, np.int64)
        e_doff = np.full(EP, 999.0, np.float32)
        e_ds = np.zeros(EP, np.float32)
        e_dd = np.zeros(EP, np.float32)
        pos = 0
        off = np.concatenate([[0], np.cumsum(cnts[c])])
        for b in range(NB):
            s, e = int(off[b]), int(off[b + 1])
            n = e - s
            e_src[pos:pos + n] = ls[s:e]
            e_doff[pos:pos + n] = (ld[s:e] - b * W).astype(np.float32)
            e_ds[pos:pos + n] = dgs[s:e]
            e_dd[pos:pos + n] = dgd[s:e]
            pos += int(padded[b])
        src_w = np.tile(e_src.astype(np.int16).reshape(EPS16, 16).T, (8, 1))
        doff_w = np.ascontiguousarray(e_doff.reshape(EPQ, 128).T)
        ds_w = np.ascontiguousarray(e_ds.reshape(EPQ, 128).T)
        dd_w = np.ascontiguousarray(e_dd.reshape(EPQ, 128).T)

        xT = np.zeros((H, NPC), np.float32)
        for s in range(GPC[c]):
            g = G0[c] + s
            xT[:, s * SLOT:s * SLOT + NPG] = x[g * NPG:(g + 1) * NPG].T

        in_maps.append({
            "xT": xT, "src_w": np.ascontiguousarray(src_w), "doff_w": doff_w,
            "ds_w": ds_w, "dd_w": dd_w,
        })

    import ml_dtypes
    mmnp = np.float32 if MM_DT != "bf16" else ml_dtypes.bfloat16
    iota3 = np.tile(np.arange(W, dtype=np.float32)[None, None, :], (128, K, 1))
    params = {
        "embW": np.ascontiguousarray(inputs["emb_W"], np.float32),
        "fcW": np.ascontiguousarray(np.asarray(inputs["fc_W"]).transpose(1, 0, 2)).astype(mmnp),
        "gammaT": np.ascontiguousarray(np.asarray(inputs["bn_gamma"]).T.astype(np.float32)),
        "betaT": np.ascontiguousarray(np.asarray(inputs["bn_beta"]).T.astype(np.float32)),
        "W1": np.ascontiguousarray(inputs["mlp_W1"], np.float32),
        "W2": np.ascontiguousarray(inputs["mlp_W2"], np.float32),
        "W3": np.ascontiguousarray(inputs["mlp_W3"], np.float32),
        "iota3": np.ascontiguousarray(iota3.astype(mmnp)),
        "emb_b": np.asarray(inputs["emb_b"], np.float32).reshape(1, H),
        "mlp_b1": np.asarray(inputs["mlp_b1"], np.float32).reshape(1, 64),
        "mlp_b2": np.asarray(inputs["mlp_b2"], np.float32).reshape(1, 32),
        "mlp_b3": np.asarray(inputs["mlp_b3"], np.float32).reshape(1, 10),
        "ones": np.ones((1, 512), mmnp),
    }
    for m in in_maps:
        m.update(params)

    small = {k: np.asarray(inputs[k], np.float32)
             for k in ("pp_W", "pp_b", "mu", "inv_sigma", "emb_b", "fc_b",
                       "mlp_b1", "mlp_b2", "mlp_b3")}
    assert np.all(small["fc_b"] == 0.0), "nonzero fc_b not supported"
    return dict(EP=EP, EPQ=EPQ, EPS16=EPS16, padded=padded, tstart=tstart,
                small=small), in_maps


def _bc(ap, reps):
    """[P, n] -> [P, n, reps] via 0-stride broadcast."""
    return AP(tensor=ap.tensor, offset=ap.offset,
              ap=[list(a) for a in ap.ap] + [[0, reps]])


def _build(meta):
    EP, EPQ, EPS16 = meta["EP"], meta["EPQ"], meta["EPS16"]
    padded, tstart = meta["padded"], meta["tstart"]
    small = meta["small"]
    mm_dt = {"f32r": F32R, "f32": F32, "bf16": BF16}[MM_DT]
    cast = lambda ap: ap
    KW = K * W

    nc = bacc.Bacc("TRN2", target_bir_lowering=False, debug=False,
                   num_devices=N_CORES)

    xT_d = nc.dram_tensor("xT", [H, NPC], mm_dt, kind="ExternalInput")
    srcw_d = nc.dram_tensor("src_w", [128, EPS16], mybir.dt.int16,
                            kind="ExternalInput")
    doff_d = nc.dram_tensor("doff_w", [128, EPQ], F32, kind="ExternalInput")
    ds_d = nc.dram_tensor("ds_w", [128, EPQ], F32, kind="ExternalInput")
    dd_d = nc.dram_tensor("dd_w", [128, EPQ], F32, kind="ExternalInput")
    embW_d = nc.dram_tensor("embW", [H, H], mm_dt, kind="ExternalInput")
    fcW_d = nc.dram_tensor("fcW", [H, L, K * H], mm_dt, kind="ExternalInput")
    gammaT_d = nc.dram_tensor("gammaT", [H, L], F32, kind="ExternalInput")
    betaT_d = nc.dram_tensor("betaT", [H, L], F32, kind="ExternalInput")
    W1_d = nc.dram_tensor("W1", [H, 64], F32, kind="ExternalInput")
    W2_d = nc.dram_tensor("W2", [64, 32], F32, kind="ExternalInput")
    W3_d = nc.dram_tensor("W3", [32, 10], F32, kind="ExternalInput")
    iota3_d = nc.dram_tensor("iota3", [128, K, W], mm_dt, kind="ExternalInput")
    embb_d = nc.dram_tensor("emb_b", [1, H], mm_dt, kind="ExternalInput")
    b1_d = nc.dram_tensor("mlp_b1", [1, 64], F32, kind="ExternalInput")
    b2_d = nc.dram_tensor("mlp_b2", [1, 32], F32, kind="ExternalInput")
    b3_d = nc.dram_tensor("mlp_b3", [1, 10], F32, kind="ExternalInput")
    ones_d = nc.dram_tensor("ones", [1, 512], mm_dt, kind="ExternalInput")
    yT_d = nc.dram_tensor("yT", [10, 8], F32, kind="ExternalOutput")
    KDBG = bool(int(os.environ.get("KDBG", "0")))
    if KDBG:
        dbg_h0 = nc.dram_tensor("dbg_h0", [H, NPC], F32, kind="ExternalOutput")
        dbg_gw = nc.dram_tensor("dbg_gw", [128, EPQ, K], mm_dt, kind="ExternalOutput")
        dbg_msg0 = nc.dram_tensor("dbg_msg0", [H, NPC], F32, kind="ExternalOutput")
        dbg_st = nc.dram_tensor("dbg_st", [H, 2], F32, kind="ExternalOutput")
        dbg_sg = nc.dram_tensor("dbg_sg", [H, 2], F32, kind="ExternalOutput")
        dbg_h1 = nc.dram_tensor("dbg_h1", [H, NPC], F32, kind="ExternalOutput")
        dbg_hg = nc.dram_tensor("dbg_hg", [128, 8, H], mm_dt, kind="ExternalOutput")
        dbg_rhs3 = nc.dram_tensor("dbg_rhs3", [128, K, W], mm_dt, kind="ExternalOutput")
        dbg_agg = nc.dram_tensor("dbg_agg", [128, K * W], mm_dt, kind="ExternalOutput")
        dbg_aggall = nc.dram_tensor("dbg_aggall", [NB, 128, K * W], mm_dt, kind="ExternalOutput")

    h_drams = [nc.dram_tensor(f"h_dram{i}", [NPC, H], mm_dt) for i in range(L + 1)]
    cc_in = [nc.dram_tensor(f"cc_in{i}", [H, 2], F32) for i in range(L)]
    cc_out = [nc.dram_tensor(f"cc_out{i}", [H, 2], F32, addr_space="Shared")
              for i in range(L)]

    ppW, ppb = small["pp_W"], small["pp_b"]
    mu, isg = small["mu"], small["inv_sigma"]

    with tile.TileContext(nc) as tc:
        from contextlib import ExitStack
        ctx = ExitStack()
        perm = ctx.enter_context(tc.tile_pool(name="perm", bufs=1))
        gwp = ctx.enter_context(tc.tile_pool(name="gwp", bufs=1))
        gwt = ctx.enter_context(tc.tile_pool(name="gwt", bufs=1))
        work = ctx.enter_context(tc.tile_pool(name="work", bufs=3))
        rhsp = ctx.enter_context(tc.tile_pool(name="rhsp", bufs=4))
        hgp = ctx.enter_context(tc.tile_pool(name="hgp", bufs=3))
        stgp = ctx.enter_context(tc.tile_pool(name="stgp", bufs=4))
        bnrp = ctx.enter_context(tc.tile_pool(name="bnrp", bufs=2))
        psA = ctx.enter_context(tc.tile_pool(name="psA", bufs=2, space="PSUM"))
        psM = ctx.enter_context(tc.tile_pool(name="psM", bufs=2, space="PSUM"))
        psT = ctx.enter_context(tc.tile_pool(name="psT", bufs=2, space="PSUM"))

        # ---- persistent tiles
        idx_sb = perm.tile([128, EPS16], mybir.dt.int16)
        nc.sync.dma_start(out=idx_sb[:], in_=srcw_d[:])
        doff_sb = perm.tile([128, EPQ], F32)
        nc.sync.dma_start(out=doff_sb[:], in_=doff_d[:])
        iota3_sb = perm.tile([128, K, W], mm_dt)
        nc.sync.dma_start(out=iota3_sb[:], in_=iota3_d[:])
        embW_sb = perm.tile([H, H], mm_dt)
        nc.sync.dma_start(out=embW_sb[:], in_=embW_d[:])
        fcW_sb = perm.tile([H, L, K * H], mm_dt)
        nc.sync.dma_start(out=fcW_sb[:], in_=fcW_d[:])
        gammaT_sb = perm.tile([H, L], F32)
        nc.sync.dma_start(out=gammaT_sb[:], in_=gammaT_d[:])
        betaT_sb = perm.tile([H, L], F32)
        nc.sync.dma_start(out=betaT_sb[:], in_=betaT_d[:])
        W1_sb = perm.tile([H, 64], F32)
        nc.sync.dma_start(out=W1_sb[:], in_=W1_d[:])
        W2_sb = perm.tile([64, 32], F32)
        nc.sync.dma_start(out=W2_sb[:], in_=W2_d[:])
        W3_sb = perm.tile([32, 10], F32)
        nc.sync.dma_start(out=W3_sb[:], in_=W3_d[:])
        embb_sb = perm.tile([1, H], mm_dt)
        nc.sync.dma_start(out=embb_sb[:], in_=embb_d[:])
        b1_sb = perm.tile([1, 64], F32)
        nc.sync.dma_start(out=b1_sb[:], in_=b1_d[:])
        b2_sb = perm.tile([1, 32], F32)
        nc.sync.dma_start(out=b2_sb[:], in_=b2_d[:])
        b3_sb = perm.tile([1, 10], F32)
        nc.sync.dma_start(out=b3_sb[:], in_=b3_d[:])
        ident = perm.tile([128, 128], F32)
        make_identity(nc, ident[:])
        ones_sb = perm.tile([1, 512], mm_dt)
        nc.sync.dma_start(out=ones_sb[:], in_=ones_d[:])
        ones32_sb = perm.tile([1, 8], F32)
        nc.vector.memset(ones32_sb[:], 1.0)

        hA = perm.tile([H, NPC], F32, tag="hA")
        hB = perm.tile([H, NPC], F32, tag="hB")

        # pseudo coords (layer-independent): ps = 1/sqrt(deg+1)
        ps0 = perm.tile([128, EPQ], F32, tag="ps0")
        ps1 = perm.tile([128, EPQ], F32, tag="ps1")
        for dsrc, pst in ((ds_d, ps0), (dd_d, ps1)):
            t = gwt.tile([128, EPQ], F32, tag="gwtmp")
            nc.sync.dma_start(out=t[:], in_=dsrc[:])
            nc.vector.tensor_scalar_add(out=t[:], in0=t[:], scalar1=1.0)
            nc.vector.reciprocal(out=t[:], in_=t[:])
            nc.scalar.sqrt(out=pst[:], in_=t[:])

        def store_hT(hT, layer):
            """transpose [H, NPC] chunks -> h_dram[layer] [NPC, H]"""
            for ci in range(NPC // 128):
                tp = psT.tile([128, 128], F32, tag="tp")
                nc.tensor.transpose(out=tp[:], in_=hT[:, ci * 128:(ci + 1) * 128],
                                    identity=ident[:])
                st = stgp.tile([128, 128], mm_dt, tag="tstage")
                nc.scalar.copy(out=st[:], in_=tp[:])
                nc.sync.dma_start(out=h_drams[layer][ci * 128:(ci + 1) * 128, :],
                                  in_=st[:])

        # ---- embedding: h0^T = embW^T @ x^T (+ emb_b)
        CH = 512 if NPC % 512 == 0 else 384
        has_embb = bool(np.any(small["emb_b"]))
        for ci in range(NPC // CH):
            sl = slice(ci * CH, (ci + 1) * CH)
            xs = work.tile([128, CH], mm_dt, tag="xchunk")
            nc.sync.dma_start(out=xs[:], in_=xT_d[:, sl])
            ph = psM.tile([128, CH], F32, tag="embps")
            nc.tensor.matmul(out=ph[:], lhsT=cast(embW_sb[:]), rhs=cast(xs[:]),
                             start=True, stop=not has_embb)
            if has_embb:
                nc.tensor.matmul(out=ph[:], lhsT=embb_sb[:], rhs=ones_sb[:],
                                 start=False, stop=True)
            nc.scalar.copy(out=hA[:, sl], in_=ph[:])
        store_hT(hA, 0)
        if KDBG:
            nc.sync.dma_start(out=dbg_h0[:], in_=hA[:])

        hT, hN = hA, hB
        # ---- layers
        for li in range(L):
            # --- edge gaussian weights gw3 [128, EPQ, K]
            p0 = gwt.tile([128, EPQ], F32, tag="p0")
            p1 = gwt.tile([128, EPQ], F32, tag="p1")
            tmp = gwt.tile([128, EPQ], F32, tag="gwtmp")
            t2 = gwt.tile([128, EPQ], F32, tag="gwtmp2")
            gw3 = gwp.tile([128, EPQ, K], mm_dt, tag="gw3")
            for j, pj in ((0, p0), (1, p1)):
                nc.vector.tensor_scalar_mul(out=tmp[:], in0=ps0[:],
                                            scalar1=float(ppW[li, 0, j]))
                nc.vector.scalar_tensor_tensor(
                    out=tmp[:], in0=ps1[:], scalar=float(ppW[li, 1, j]),
                    in1=tmp[:], op0=OP.mult, op1=OP.add)
                nc.scalar.activation(out=pj[:], in_=tmp[:], func=AF.Tanh,
                                     bias=float(ppb[li, j]))
            for k in range(K):
                nc.vector.tensor_scalar(
                    out=tmp[:], in0=p0[:],
                    scalar1=float(isg[li, k, 0]),
                    scalar2=float(mu[li, k, 0] * isg[li, k, 0]),
                    op0=OP.mult, op1=OP.subtract)
                nc.vector.tensor_mul(out=tmp[:], in0=tmp[:], in1=tmp[:])
                nc.vector.tensor_scalar(
                    out=t2[:], in0=p1[:],
                    scalar1=float(isg[li, k, 1]),
                    scalar2=float(mu[li, k, 1] * isg[li, k, 1]),
                    op0=OP.mult, op1=OP.subtract)
                nc.vector.tensor_mul(out=t2[:], in0=t2[:], in1=t2[:])
                nc.vector.tensor_add(out=tmp[:], in0=tmp[:], in1=t2[:])
                nc.scalar.activation(out=gw3[:, :, k], in_=tmp[:], func=AF.Exp,
                                     scale=-0.5)
            if KDBG and li == 0:
                nc.sync.dma_start(out=dbg_gw[:], in_=gw3[:])

            # --- gather + aggregate + fc + stats
            msg = hN
            sums = work.tile([128, NB], F32, tag="sums")
            sqs = work.tile([128, NB], F32, tag="sqs")
            sqtmp = work.tile([128, W], F32, tag="sqtmp")
            hg = None
            for b in range(NB):
                nt = int(padded[b]) // 128
                agg = psA.tile([128, KW], F32, tag="agg")
                for j in range(nt):
                    et = int(tstart[b]) + j
                    if et % 8 == 0:
                        hg = hgp.tile([128, 8, H], mm_dt, tag="hg")
                        nc.gpsimd.dma_gather(
                            out_ap=hg[:], in_ap=h_drams[li][:],
                            idxs_ap=idx_sb[:, et * 8:(et + 8) * 8],
                            num_idxs=GCHUNK, num_idxs_reg=GCHUNK, elem_size=H)
                    rhs3 = rhsp.tile([128, K, W], mm_dt, tag="rhs3")
                    nc.vector.scalar_tensor_tensor(
                        out=rhs3[:], in0=iota3_sb[:],
                        scalar=doff_sb[:, et:et + 1],
                        in1=_bc(gw3[:, et, :], W),
                        op0=OP.is_equal, op1=OP.mult)
                    if KDBG and li == 0 and et == 0:
                        nc.sync.dma_start(out=dbg_hg[:], in_=hg[:])
                        nc.sync.dma_start(out=dbg_rhs3[:], in_=rhs3[:])
                    nc.tensor.matmul(
                        out=agg[:],
                        lhsT=cast(hg[:, et % 8, :]),
                        rhs=cast(rhs3[:].rearrange("p k w -> p (k w)")),
                        start=(j == 0), stop=(j == nt - 1))
                agg_sb = work.tile([128, KW], mm_dt, tag="aggsb")
                nc.scalar.copy(out=agg_sb[:], in_=agg[:])
                if KDBG and li == 0 and b == 0:
                    nc.sync.dma_start(out=dbg_agg[:], in_=agg_sb[:])
                if KDBG and li == 0:
                    nc.sync.dma_start(out=dbg_aggall[b], in_=agg_sb[:])
                mps = psM.tile([128, W], F32, tag="msgps")
                for k in range(K):
                    nc.tensor.matmul(
                        out=mps[:],
                        lhsT=cast(fcW_sb[:, li, k * H:(k + 1) * H]),
                        rhs=cast(agg_sb[:, k * W:(k + 1) * W]),
                        start=(k == 0), stop=(k == K - 1))
                bsl = slice(b * W, (b + 1) * W)
                nc.scalar.activation(out=msg[:, bsl], in_=mps[:], func=AF.Copy,
                                     accum_out=sums[:, b:b + 1])
                nc.vector.scalar_tensor_tensor(
                    out=sqtmp[:], in0=msg[:, bsl], scalar=1.0, in1=msg[:, bsl],
                    op0=OP.mult, op1=OP.mult, accum_out=sqs[:, b:b + 1])

            if KDBG and li == 0:
                nc.sync.dma_start(out=dbg_msg0[:], in_=msg[:])
            # --- BN stats allreduce
            st2 = work.tile([128, 2], F32, tag="stats")
            nc.vector.tensor_reduce(out=st2[:, 0:1], in_=sums[:], axis=AX.X,
                                    op=OP.add)
            nc.vector.tensor_reduce(out=st2[:, 1:2], in_=sqs[:], axis=AX.X,
                                    op=OP.add)
            nc.sync.dma_start(out=cc_in[li][:], in_=st2[:])
            nc.gpsimd.collective_compute(
                "AllReduce", OP.add, replica_groups=[list(range(N_CORES))],
                ins=[cc_in[li][:]], outs=[cc_out[li][:]])
            sg2 = work.tile([128, 2], F32, tag="stats2")
            nc.sync.dma_start(out=sg2[:], in_=cc_out[li][:])
            if KDBG and li == 0:
                nc.sync.dma_start(out=dbg_st[:], in_=st2[:])
                nc.sync.dma_start(out=dbg_sg[:], in_=sg2[:])
            mean = work.tile([128, 1], F32, tag="mean")
            nc.vector.tensor_scalar_mul(out=mean[:], in0=sg2[:, 0:1],
                                        scalar1=1.0 / N)
            var = work.tile([128, 1], F32, tag="var")
            nc.vector.scalar_tensor_tensor(
                out=var[:], in0=mean[:], scalar=-1.0, in1=mean[:],
                op0=OP.mult, op1=OP.mult)
            nc.vector.scalar_tensor_tensor(
                out=var[:], in0=sg2[:, 1:2], scalar=1.0 / N, in1=var[:],
                op0=OP.mult, op1=OP.add)
            nc.vector.tensor_scalar_add(out=var[:], in0=var[:], scalar1=EPS_BN)
            rstd = work.tile([128, 1], F32, tag="rstd")
            nc.scalar.sqrt(out=rstd[:], in_=var[:])
            nc.vector.reciprocal(out=rstd[:], in_=rstd[:])
            scl = work.tile([128, 1], F32, tag="scl")
            nc.vector.tensor_mul(out=scl[:], in0=rstd[:],
                                 in1=gammaT_sb[:, li:li + 1])
            bias = work.tile([128, 1], F32, tag="bias")
            nc.vector.scalar_tensor_tensor(
                out=bias[:], in0=mean[:], scalar=-1.0, in1=scl[:],
                op0=OP.mult, op1=OP.mult)
            nc.vector.tensor_add(out=bias[:], in0=bias[:],
                                 in1=betaT_sb[:, li:li + 1])

            # --- bn+relu+residual into msg (becomes new h)
            for ci in range(NPC // CH):
                sl = slice(ci * CH, (ci + 1) * CH)
                bnr = bnrp.tile([128, CH], F32, tag="bnr")
                nc.scalar.activation(out=bnr[:], in_=msg[:, sl], func=AF.Relu,
                                     bias=bias[:, 0:1], scale=scl[:, 0:1])
                nc.vector.tensor_add(out=msg[:, sl], in0=bnr[:], in1=hT[:, sl])
            if li < L - 1:
                store_hT(msg, li + 1)
            if KDBG and li == 0:
                nc.sync.dma_start(out=dbg_h1[:], in_=msg[:])
            hT, hN = msg, hT

        # ---- readout
        hgr = work.tile([128, 8], F32, tag="hgr")
        nc.vector.memset(hgr[:], 0.0)
        for s in range(MAXG):
            nc.vector.tensor_reduce(out=hgr[:, s:s + 1],
                                    in_=hT[:, s * SLOT:s * SLOT + NPG],
                                    axis=AX.X, op=OP.add)
        nc.scalar.mul(out=hgr[:], in_=hgr[:], mul=1.0 / NPG)
        y1p = psM.tile([64, 8], F32, tag="embps")
        nc.tensor.matmul(out=y1p[:], lhsT=W1_sb[:], rhs=hgr[:], start=True,
                         stop=not np.any(small["mlp_b1"]))
        if np.any(small["mlp_b1"]):
            nc.tensor.matmul(out=y1p[:], lhsT=b1_sb[:], rhs=ones32_sb[:],
                             start=False, stop=True)
        y1 = work.tile([64, 8], F32, tag="y1")
        nc.scalar.activation(out=y1[:], in_=y1p[:], func=AF.Relu)
        y2p = psM.tile([32, 8], F32, tag="embps")
        nc.tensor.matmul(out=y2p[:], lhsT=W2_sb[:], rhs=y1[:], start=True,
                         stop=not np.any(small["mlp_b2"]))
        if np.any(small["mlp_b2"]):
            nc.tensor.matmul(out=y2p[:], lhsT=b2_sb[:], rhs=ones32_sb[:],
                             start=False, stop=True)
        y2 = work.tile([32, 8], F32, tag="y2")
        nc.scalar.activation(out=y2[:], in_=y2p[:], func=AF.Relu)
        y3p = psM.tile([10, 8], F32, tag="embps")
        nc.tensor.matmul(out=y3p[:], lhsT=W3_sb[:], rhs=y2[:], start=True,
                         stop=not np.any(small["mlp_b3"]))
        if np.any(small["mlp_b3"]):
            nc.tensor.matmul(out=y3p[:], lhsT=b3_sb[:], rhs=ones32_sb[:],
                             start=False, stop=True)
        y3 = work.tile([10, 8], F32, tag="y3")
        nc.scalar.copy(out=y3[:], in_=y3p[:])
        nc.sync.dma_start(out=yT_d[:], in_=y3[:])

        ctx.close()

    nc.compile()
    return nc


def kernel(**inputs):
    meta, in_maps = _prepare(inputs)
    nc = _build(meta)
    res = run_bass_kernel_spmd(nc, in_maps, list(range(N_CORES)),
                               trace=bool(int(os.environ.get("KTRACE", "0"))))
    y = np.zeros((G, 10), np.float32)
    for c in range(N_CORES):
        yT = res.results[c]["yT"]
        y[G0[c]:G0[c + 1]] = yT[:, :GPC[c]].T
    if os.environ.get("KTRACE", "0") != "0":
        kernel.last_exec_time_ns = res.exec_time_ns
        kernel.last_profile = res.profile_json
    return y
